# revision 1
# baseline (speedup 1.0000x reference)
"""Trainium2 Bass kernel for CustomLSTMModel.

Model: tokens [256,512] -> embedding (padding_idx=0) -> 1-layer LSTM(300->512)
       -> last hidden state -> FC(512->7).

Strategy (8 NeuronCores, data-parallel over batch, 32 rows/core):
  - Fused per-step gate matmul: gates = [x_t; h_{t-1}; 1] @ [W_ihT; W_hhT; b]
    as 7 K-rounds of 4 concurrent M=32 matmuls packed into the PE array's
    four 32-column groups (tile_position), N=512 each.
  - Weight columns are permuted host-side so PSUM partition-block j holds
    [i|f|o|g] gate slices for hidden-quarter j: all elementwise ops run on
    full 128-partition tiles, batch=32 x quarter=4 on partitions.
  - sigmoid/tanh on ACT directly from PSUM; c/h updates on DVE in fp32;
    h recast to bf16 and transposed back to lhsT layout via 4 PE transposes.
  - Embedding rows are gathered 128 at a time (4 steps) by indirect DMA and
    transposed to K-major via PE transposes.
All matmul inputs bf16, PSUM accumulation fp32, cell state fp32.
"""
import numpy as np
import ml_dtypes

import concourse.bass as bass
import concourse.tile as tile
from concourse import bacc, mybir
from concourse.bass_utils import run_bass_kernel_spmd

BF16 = mybir.dt.bfloat16
F32 = mybir.dt.float32
I32 = mybir.dt.int32

B, S, E, H, OUT = 256, 512, 300, 512, 7
NCORES = 8
BC = B // NCORES          # batch per core (32)
NG = S // 4               # token groups of 128 = 4 steps
PREFETCH = 2              # gather groups in flight ahead

_BUILD_CACHE = {}


def _build(n_steps=S, amp=1):
    """Build + compile the Bass program for one core (SPMD across 8).

    amp > 1 repeats the whole LSTM pass on-device (perf measurement only).
    """
    if (n_steps, amp) in _BUILD_CACHE:
        return _BUILD_CACHE[(n_steps, amp)]
    ngroups = (n_steps + 3) // 4
    nc = bacc.Bacc("TRN2", target_bir_lowering=False, debug=False)

    wcat = nc.dram_tensor("wcat", [7, 128, 2048], BF16, kind="ExternalInput")
    emb = nc.dram_tensor("emb", [32000, E], BF16, kind="ExternalInput")
    toks = nc.dram_tensor("toks", [NG, 128, 1], I32, kind="ExternalInput")
    ident = nc.dram_tensor("ident", [128, 32], BF16, kind="ExternalInput")
    identx = nc.dram_tensor("identx", [128, 128], BF16, kind="ExternalInput")
    wfct = nc.dram_tensor("wfct", [4, 128, OUT], BF16, kind="ExternalInput")
    bfc = nc.dram_tensor("bfc", [1, OUT], BF16, kind="ExternalInput")
    logits = nc.dram_tensor("logits", [BC, OUT], F32, kind="ExternalOutput")

    SIG = mybir.ActivationFunctionType.Sigmoid
    TANH = mybir.ActivationFunctionType.Tanh

    with tile.TileContext(nc) as tc:
        with (
            tc.tile_pool(name="const", bufs=1) as cpool,
            tc.tile_pool(name="xg", bufs=PREFETCH + 2) as xpool,
            tc.tile_pool(name="state", bufs=2) as spool,
            tc.tile_pool(name="work", bufs=2) as wpool,
            tc.tile_pool(name="gpsum", bufs=2, space="PSUM") as gpsum,
            tc.tile_pool(name="tpsum", bufs=1, space="PSUM") as tpsum,
            tc.tile_pool(name="xpsum", bufs=2, space="PSUM") as xpsum,
        ):
            # ---- constants ----
            wcat_sb = []
            for r in range(7):
                wt = cpool.tile([128, 2048], BF16, tag=f"wcat{r}")
                nc.sync.dma_start(wt[:], wcat.ap()[r])
                wcat_sb.append(wt)
            ident_sb = cpool.tile([128, 32], BF16, tag="ident")
            nc.sync.dma_start(ident_sb[:], ident.ap())
            identx_sb = cpool.tile([128, 128], BF16, tag="identx")
            nc.sync.dma_start(identx_sb[:], identx.ap())
            wfct_sb = []
            for k in range(4):
                wf = cpool.tile([128, OUT], BF16, tag=f"wfct{k}")
                nc.sync.dma_start(wf[:], wfct.ap()[k])
                wfct_sb.append(wf)
            bfc_sb = cpool.tile([1, OUT], BF16, tag="bfc")
            nc.sync.dma_start(bfc_sb[:], bfc.ap())
            ones_sb = cpool.tile([1, 32], BF16, tag="ones")
            nc.gpsimd.memset(ones_sb[:], 1.0)

            c_sb = cpool.tile([128, 128], F32, tag="cstate")
            nc.gpsimd.memset(c_sb[:], 0.0)

            # ---- x pipeline: gather 128 emb rows -> transpose to K-major ----
            def prefetch(g):
                tok_sb = xpool.tile([128, 1], I32, tag="tok")
                nc.sync.dma_start(tok_sb[:], toks.ap()[g])
                x_sb = xpool.tile([128, 304], BF16, tag="xsb")
                nc.gpsimd.memset(x_sb[:, 300:301], 1.0)
                nc.gpsimd.indirect_dma_start(
                    out=x_sb[:, 0:E],
                    out_offset=None,
                    in_=emb.ap(),
                    in_offset=bass.IndirectOffsetOnAxis(ap=tok_sb[:, :1], axis=0),
                )
                xp = xpsum.tile([128, 384], BF16, tag="xp")
                for s_i in range(3):
                    w = min(128, 301 - 128 * s_i)  # 128,128,45 (45th = ones)
                    nc.tensor.transpose(
                        out=xp[0:w, 128 * s_i:128 * s_i + 128],
                        in_=x_sb[:, 128 * s_i:128 * s_i + w],
                        identity=identx_sb[:],
                        tile_position=(0, 0),
                    )
                xg = xpool.tile([128, 384], BF16, tag="xgall")
                nc.vector.tensor_copy(xg[:, 0:256], xp[:, 0:256])
                nc.vector.tensor_copy(xg[0:45, 256:384], xp[0:45, 256:384])
                return xg

            hT_all = None
            for rep in range(amp):
              xg_tiles = {}
              for g in range(min(PREFETCH, ngroups)):
                  xg_tiles[g] = prefetch(g)

              def emit_xr(t, first):
                  """x-projection rounds of step t into a fresh gates tile."""
                  g, lt = t // 4, t % 4
                  if lt == 0 and g + PREFETCH < ngroups:
                      xg_tiles[g + PREFETCH] = prefetch(g + PREFETCH)
                  xg = xg_tiles[g]
                  gates = gpsum.tile([128, 512], F32, tag="gates")
                  rounds = [
                      (xg[0:128, 0 + 32 * lt:0 + 32 * lt + 32], wcat_sb[0][:]),
                      (xg[0:128, 128 + 32 * lt:128 + 32 * lt + 32], wcat_sb[1][:]),
                      (xg[0:45, 256 + 32 * lt:256 + 32 * lt + 32], wcat_sb[2][0:45, :]),
                  ]
                  for r, (lh, wt) in enumerate(rounds):
                      for j in range(4):
                          nc.tensor.matmul(
                              out=gates[32 * j:32 * (j + 1), :], lhsT=lh,
                              rhs=wt[:, 512 * j:512 * (j + 1)],
                              start=(r == 0), stop=(first and r == 2),
                              tile_position=(0, 32 * j), skip_group_check=True)
                  return gates

              gates_q = {0: emit_xr(0, first=(rep == 0 or True) and hT_all is None)}
              for t in range(n_steps):
                gates = gates_q.pop(t)
                # ---- recurrent rounds (stacked on the x rounds) ----
                if hT_all is not None:
                    for k in range(4):
                        for j in range(4):
                            nc.tensor.matmul(
                                out=gates[32 * j:32 * (j + 1), :],
                                lhsT=hT_all[:, 32 * k:32 * k + 32],
                                rhs=wcat_sb[3 + k][:, 512 * j:512 * (j + 1)],
                                start=False, stop=(k == 3),
                                tile_position=(0, 32 * j), skip_group_check=True)

                # ---- elementwise: [i|f|o] sigmoid, g tanh, LSTM cell update ----
                sg = wpool.tile([128, 384], F32, tag="sig")
                nc.scalar.activation(out=sg[:], in_=gates[:, 0:384], func=SIG)
                gg = wpool.tile([128, 128], F32, tag="gtanh")
                nc.scalar.activation(out=gg[:], in_=gates[:, 384:512], func=TANH)
                tmp = wpool.tile([128, 128], F32, tag="tmp")
                nc.gpsimd.tensor_tensor(out=tmp[:], in0=sg[:, 0:128], in1=gg[:],
                                        op=mybir.AluOpType.mult)
                cnew = wpool.tile([128, 128], F32, tag="cnew")
                nc.vector.tensor_tensor(out=cnew[:], in0=sg[:, 128:256], in1=c_sb[:],
                                        op=mybir.AluOpType.mult)
                nc.vector.tensor_tensor(out=c_sb[:], in0=cnew[:], in1=tmp[:],
                                        op=mybir.AluOpType.add)
                tc_t = wpool.tile([128, 128], F32, tag="tanhc")
                nc.scalar.activation(out=tc_t[:], in_=c_sb[:], func=TANH)
                hw_t = wpool.tile([128, 128], BF16, tag="hwide")
                nc.vector.tensor_tensor(out=hw_t[:], in0=sg[:, 256:384], in1=tc_t[:],
                                        op=mybir.AluOpType.mult)

                # next step's x rounds BEFORE this step's transposes: the PE
                # executes in order, so this fills the elementwise-chain gap
                if t + 1 < n_steps:
                    gates_q[t + 1] = emit_xr(t + 1, first=False)

                # ---- transpose h back to lhsT (K-major) layout ----
                # one PSUM bank per transpose: concurrent PE writes into one
                # bank on the same partitions crash the device
                hT_all = spool.tile([128, 128], BF16, tag="hT")
                for j in range(4):
                    hp = tpsum.tile([128, 32], BF16, tag=f"hp{j}")
                    nc.tensor.transpose(
                        out=hp[:],
                        in_=hw_t[32 * j:32 * (j + 1), :],
                        identity=ident_sb[32 * j:32 * (j + 1), :],
                        tile_position=(32 * j, 0),
                    )
                    nc.vector.tensor_copy(hT_all[:, 32 * j:32 * (j + 1)], hp[:])

            # ---- FC head: logits = h_T @ W_fc.T + b_fc ----
            fc_ps = gpsum.tile([32, OUT], F32, tag="gates")
            for k in range(4):
                nc.tensor.matmul(out=fc_ps[:], lhsT=hT_all[:, 32 * k:32 * k + 32],
                                 rhs=wfct_sb[k][:], start=(k == 0), stop=False,
                                 tile_position=(0, 0))
            nc.tensor.matmul(out=fc_ps[:], lhsT=ones_sb[:], rhs=bfc_sb[:],
                             start=False, stop=True, tile_position=(0, 0))
            fc_sb = wpool.tile([32, OUT], F32, tag="fcout")
            nc.scalar.copy(out=fc_sb[:], in_=fc_ps[:])
            nc.sync.dma_start(logits.ap(), fc_sb[:])

    nc.compile()
    _BUILD_CACHE[(n_steps, amp)] = nc
    return nc


def _prep_inputs(tokens, emb, W_ih, b_ih, W_hh, b_hh, W_fc, b_fc, n_steps=S):
    """Host-side weight packing (dtype casts, transposes, gate permutation)."""
    bf = ml_dtypes.bfloat16
    # gate column permutation: our col n = 128*(4j+gs)+jr maps to orig
    # 512*go+128j+jr with gs order [i,f,o,g] -> orig gate idx [0,1,3,2]
    perm = np.empty(2048, np.int64)
    go_of_gs = [0, 1, 3, 2]
    for j in range(4):
        for gs in range(4):
            base = 128 * (4 * j + gs)
            perm[base:base + 128] = 512 * go_of_gs[gs] + 128 * j + np.arange(128)

    WihT = W_ih.T.astype(np.float32)[:, perm]          # [300, 2048]
    WhhT = W_hh.T.astype(np.float32)[:, perm]          # [512, 2048]
    bias = (b_ih + b_hh).astype(np.float32)[perm]      # [2048]

    wcat = np.zeros((7, 128, 2048), np.float32)
    wcat[0] = WihT[0:128]
    wcat[1] = WihT[128:256]
    wcat[2][0:44] = WihT[256:300]
    wcat[2][44] = bias
    for k in range(4):
        wcat[3 + k] = WhhT[128 * k:128 * (k + 1)]
    wcat = wcat.astype(bf)

    emb0 = emb.astype(np.float32).copy()
    emb0[0] = 0.0
    emb_bf = emb0.astype(bf)

    ident = np.tile(np.eye(32, dtype=bf), (4, 1))          # [128, 32]
    identx = np.eye(128, dtype=bf)
    wfct = np.ascontiguousarray(
        W_fc.T.astype(np.float32).reshape(4, 128, OUT)).astype(bf)
    bfc = b_fc.astype(np.float32).reshape(1, OUT).astype(bf)

    ngroups = (n_steps + 3) // 4
    in_maps = []
    for core in range(NCORES):
        tcore = tokens[core * BC:(core + 1) * BC]          # [32, 512]
        tg = np.ascontiguousarray(tcore.T)                 # [512, 32] (t, b)
        tg = tg.reshape(NG, 4 * BC, 1).astype(np.int32)    # [(g), (lt,b), 1]
        in_maps.append({
            "wcat": wcat, "emb": emb_bf, "toks": tg,
            "ident": ident, "identx": identx,
            "wfct": wfct, "bfc": bfc,
        })
    return in_maps


def kernel(tokens, emb, W_ih, b_ih, W_hh, b_hh, W_fc, b_fc, n_steps=S,
           profile=False):
    nc = _build(n_steps)
    in_maps = _prep_inputs(tokens, emb, W_ih, b_ih, W_hh, b_hh, W_fc, b_fc,
                           n_steps=n_steps)
    kw = {}
    if profile:
        kw = dict(trace=True, tmpdir="/tmp/lstm_trace")
    res = run_bass_kernel_spmd(nc, in_maps, list(range(NCORES)), **kw)
    out = np.concatenate([res.results[i]["logits"] for i in range(NCORES)], axis=0)
    if profile:
        kernel.last_exec_time_ns = res.exec_time_ns
        kernel.last_results = res
    return out.astype(np.float32)

